# revision 13
# baseline (speedup 1.0000x reference)
"""Multi-head attention + residual + LayerNorm, tensor-parallel over heads
across 8 Trainium2 NeuronCores.

Reference computation (SEQ=2048, BATCH=2, D_MODEL=1024, H=16, D=64):
    qkv = h @ Wqkv.T ; per-(b,head) softmax((q k^T)/8, key-padding mask) @ v
    out = LayerNorm(h + concat_heads @ Wo.T) * gamma + beta

Sharding: 2 heads per core. Per core: QKV projection for its heads (bf16),
attention with scores kept transposed [key, query]; softmax exp is spread
across all three elementwise engines: ScalarE runs real exp, VectorE/Pool
run a Schraudolph bit-trick exp (uint16(A*s + B) read back as bf16, ~3%
elementwise error, well inside tolerance); denominator Z comes from a
ones-column appended to v. The AllToAll redistribution is split per batch
so b=0's collective overlaps b=1's attention and b=0's Wo+LayerNorm
overlaps b=1's collective.
"""
import sys

if "/opt/trn_rl_repo" not in sys.path:
    sys.path.insert(0, "/opt/trn_rl_repo")

import numpy as np
import ml_dtypes

import bass_rust
import concourse.bass as bass
import concourse.mybir as mybir
import concourse.tile as tile
from concourse.bass_utils import run_bass_kernel_spmd

BF16 = ml_dtypes.bfloat16
F32 = mybir.dt.float32
BF = mybir.dt.bfloat16
U16 = mybir.dt.uint16

SEQ, BATCH, DM = 2048, 2, 1024
NH, DH = 16, 64
NC_ = 8                      # cores
HPC = NH // NC_              # heads per core (2)
ROWS = SEQ * BATCH // NC_    # flat rows per core (512)
RPB = ROWS // BATCH          # rows per core per batch (256)
NT = SEQ // 128              # key tiles (16)
NP = SEQ // 512              # query panels per batch (4)
LN_EPS = 1e-5
NEG = -1e30

# Schraudolph-exp constants: uint16(A*score + C) bits == bf16(exp(score/8))
SCH_A = float(np.log2(np.e) * 128.0 / np.sqrt(DH))
SCH_C = 127.0 * 128.0

# engine rotation for the exp/convert of each key tile (Pool/GpSimd cannot
# read PSUM on TRN2, so only ScalarE and VectorE take exp tiles)
EXP_PATTERN = ("act", "vec", "act")


# ---------------------------------------------------------------------------
# walrus compat: this toolchain accepts at most ONE sync-wait per instruction.
# Split excess waits (and >1 updates on non-DMA instructions) onto adjacent
# same-engine NOPs after Tile scheduling.
# ---------------------------------------------------------------------------
_DMAISH = ("InstDMACopy", "InstDMATranspose", "DmaTranspose", "InstCollectiveCompute")


def _legalize_waits(nc: bass.Bass) -> int:
    n = 0
    for fn in nc.m.functions:
        for bb in fn.blocks:
            insts = bb.instructions
            i = 0
            while i < len(insts):
                inst = insts[i]
                si = inst.sync_info
                if si is None:
                    i += 1
                    continue
                waits = list(si.on_wait)
                updates = list(si.on_update)
                is_dma = any(k in type(inst).__name__ for k in _DMAISH)
                split_upd = (not is_dma) and len(updates) > 1
                if len(waits) <= 1 and not split_upd:
                    i += 1
                    continue
                keep_u = updates if not split_upd else updates[:1]
                extra_u = [] if not split_upd else updates[1:]
                eng = nc.engines[inst.engine]
                before = []
                for w in waits[1:]:
                    nop = eng.nop(nofuse=True).ins
                    _pop_last(nc, nop)
                    nop.sync_info = bass_rust.SyncInfo(on_wait=[w], on_update=[])
                    before.append(nop)
                after = []
                for u in extra_u:
                    nop = eng.nop(nofuse=True).ins
                    _pop_last(nc, nop)
                    nop.sync_info = bass_rust.SyncInfo(on_wait=[], on_update=[u])
                    after.append(nop)
                inst.sync_info = bass_rust.SyncInfo(on_wait=waits[:1], on_update=keep_u)
                insts[i:i + 1] = before + [inst] + after
                n += len(before) + len(after)
                i += len(before) + 1 + len(after)
    return n


def _pop_last(nc, inst):
    for fn in nc.m.functions:
        for bb in fn.blocks:
            lst = bb.instructions
            if lst and lst[-1] is inst:
                lst.pop()
                return
    for fn in nc.m.functions:
        for bb in fn.blocks:
            lst = bb.instructions
            for k in range(len(lst) - 1, -1, -1):
                if lst[k] is inst:
                    del lst[k]
                    return
    raise RuntimeError("fresh nop not found")


# ---------------------------------------------------------------------------
# kernel graph
# ---------------------------------------------------------------------------
def _build(masked_full, any_mixed):
    """masked_full: set of (t, b) key-tiles fully masked (skipped entirely).
    any_mixed: whether partially-masked tiles exist (those go to ScalarE,
    whose exp bias handles the mask)."""
    nc = bass.Bass()

    hT = nc.declare_dram_parameter("hT", [BATCH, DM, SEQ], BF, isOutput=False)
    wT = nc.declare_dram_parameter("wT", [DM, 3 * HPC * DH], BF, isOutput=False)
    woT = nc.declare_dram_parameter("woT", [DM, DM], BF, isOutput=False)
    hrows = nc.declare_dram_parameter("hrows", [BATCH, RPB, DM], F32, isOutput=False)
    mb = nc.declare_dram_parameter("mb", [128, NT * BATCH], F32, isOutput=False)
    gamma = nc.declare_dram_parameter("gamma", [DM], F32, isOutput=False)
    beta = nc.declare_dram_parameter("beta", [DM], F32, isOutput=False)
    out = nc.declare_dram_parameter("out", [ROWS, DM], F32, isOutput=True)

    a2a_in = [nc.dram_tensor(f"a2a_in{b}", [NC_, 128, RPB], BF) for b in range(BATCH)]
    a2a_out = [nc.dram_tensor(f"a2a_out{b}", [NC_, 128, RPB], BF) for b in range(BATCH)]
    zb = nc.dram_tensor("zb", [BATCH, HPC, NP, 512], F32)
    zb2 = nc.dram_tensor("zb2", [BATCH, HPC, NP, 512], F32)

    unmasked = {b: [t for t in range(NT) if (t, b) not in masked_full] for b in range(BATCH)}

    with tile.TileContext(nc) as tc:
        with tc.tile_pool(name="big", bufs=1) as big, \
             tc.tile_pool(name="epool", bufs=4) as epool, \
             tc.tile_pool(name="small", bufs=4) as small, \
             tc.tile_pool(name="mm_ps", bufs=2, space="PSUM") as mm_ps, \
             tc.tile_pool(name="sc_ps", bufs=2, space="PSUM") as sc_ps, \
             tc.tile_pool(name="u_ps", bufs=1, space="PSUM") as u_ps:

            # ---- persistent SBUF tensors ----
            wt_sb = big.tile([128, 8, 3 * HPC * DH], BF)      # Wqkv shard^T chunks
            wot_sb = big.tile([128, 8, DM], BF)               # Wo^T chunks (full)
            qT = big.tile([128, BATCH, SEQ], BF)
            kT = big.tile([128, BATCH, SEQ], BF)
            vT = big.tile([128, BATCH, SEQ], BF)
            v_sb = big.tile([128, BATCH, NT, 160], BF)        # [v_h0|1|pad|v_h1|1|pad]
            vecT = big.tile([128, SEQ * BATCH], BF)           # flat (s,b) columns
            mb_sb = big.tile([128, NT * BATCH], F32)
            eps_sb = big.tile([128, 1], F32)
            gam_sb = big.tile([128, DM], F32)
            bet_sb = big.tile([128, DM], F32)
            vt_b = [big.tile([128, NC_, RPB], BF, name=f"vt_b{b}") for b in range(BATCH)]

            # ---- P1: QKV projection  qkvT = Wshard @ h^T ----
            # weight/activation chunk loads interleaved so matmuls start early
            CH = HPC * DH  # 128 channels per ch-tile (q, k, v)
            dest = (qT, kT, vT)
            ht_store = {}
            for b in range(BATCH):
                hts = []
                for m in range(8):
                    htt = big.tile([128, SEQ], BF, name=f"ht{m}", tag=f"ht{m}")
                    nc.sync.dma_start(out=htt, in_=hT[b, m * 128:(m + 1) * 128, :])
                    if b == 0:
                        nc.sync.dma_start(out=wt_sb[:, m, :], in_=wT[m * 128:(m + 1) * 128, :])
                    hts.append(htt)
                ht_store[b] = hts
                if b == 0:
                    nc.vector.memset(eps_sb, LN_EPS)
                    nc.vector.memset(v_sb[:, :, :, 64:65], 1.0)
                    nc.vector.memset(v_sb[:, :, :, 144:145], 1.0)
                    nc.sync.dma_start(out=mb_sb, in_=mb[:, :])
                    nc.sync.dma_start(out=gam_sb, in_=bass.AP(tensor=gamma, offset=0, ap=[[0, 128], [1, DM]]))
                    nc.sync.dma_start(out=bet_sb, in_=bass.AP(tensor=beta, offset=0, ap=[[0, 128], [1, DM]]))
                    for m in range(8):
                        nc.sync.dma_start(out=wot_sb[:, m, :], in_=woT[m * 128:(m + 1) * 128, :])
                for ct in range(3):
                    for p in range(NP):
                        ps = mm_ps.tile([128, 512], F32, name="ps", tag="mmps")
                        for m in range(8):
                            nc.tensor.matmul(
                                ps,
                                lhsT=wt_sb[:, m, ct * CH:(ct + 1) * CH],
                                rhs=hts[m][:, p * 512:(p + 1) * 512],
                                start=(m == 0), stop=(m == 7),
                            )
                        nc.any.tensor_copy(dest[ct][:, b, p * 512:(p + 1) * 512], ps)

                # ---- P2(b): v transpose (DMA xbar) into packed [keys, d] tile ----
                for h in range(HPC):
                    vpk = big.tile([128, NT, 64], BF, name="vpk", tag="vpk", bufs=2)
                    nc.sync.dma_start(out=vpk, in_=vT[h * 64:(h + 1) * 64, b, :], transpose=True)
                    nc.gpsimd.tensor_copy(v_sb[:, b, :, 80 * h:80 * h + 64], vpk)

            # ---- P3 + per-batch A2A + P5 pipeline ----
            vec3 = vecT.rearrange("c (s b) -> c s b", b=BATCH)
            exp_i = 0
            for b in range(BATCH):
                um = unmasked[b]
                for p in range(NP):
                    ups = [u_ps.tile([65, 512], F32, name=f"ups{h}", tag=f"ups{h}") for h in range(HPC)]
                    for ti, t in enumerate(um):
                        sc = sc_ps.tile([128, 1024], F32)
                        for h in range(HPC):
                            nc.tensor.matmul(
                                sc[:, h * 512:(h + 1) * 512],
                                lhsT=kT[h * 64:(h + 1) * 64, b, t * 128:(t + 1) * 128],
                                rhs=qT[h * 64:(h + 1) * 64, b, p * 512:(p + 1) * 512],
                                start=True, stop=True,
                            )
                        # exp(score/8 + mask): rotate across engines. Partially
                        # masked tiles must go to ScalarE (exp bias handles them).
                        partial = any_mixed and bool(
                            _partial_mask_tiles and (t, b) in _partial_mask_tiles)
                        eng = "act" if partial else EXP_PATTERN[exp_i % len(EXP_PATTERN)]
                        exp_i += 1
                        e = epool.tile([128, 1024], U16, name="e", tag="e")
                        if eng == "act":
                            nc.scalar.activation(
                                out=e.bitcast(BF)[:, :], in_=sc,
                                func=mybir.ActivationFunctionType.Exp,
                                bias=mb_sb[:, t * BATCH + b:t * BATCH + b + 1],
                                scale=1.0 / np.sqrt(DH),
                            )
                        else:
                            nc.vector.tensor_scalar(
                                out=e, in0=sc, scalar1=SCH_A, scalar2=SCH_C,
                                op0=mybir.AluOpType.mult, op1=mybir.AluOpType.add,
                            )
                        for h in range(HPC):
                            nc.tensor.matmul(
                                ups[h],
                                lhsT=v_sb[:, b, t, 80 * h:80 * h + 65],
                                rhs=e.bitcast(BF)[:, h * 512:(h + 1) * 512],
                                start=(ti == 0), stop=(ti == len(um) - 1),
                            )
                    # Z-divide, write vec^T columns (flat order s*2+b)
                    for h in range(HPC):
                        # Copy U_aug out of PSUM promptly so the next panel's
                        # matmuls can start; finish Z-divide from SBUF.
                        u_sb = small.tile([65, 512], F32, name="u_sb", tag="u_sb", bufs=4)
                        nc.any.tensor_copy(u_sb, ups[h])
                        zoff = ((b * HPC + h) * NP + p) * 512
                        nc.sync.dma_start(out=zb[b, h, p, :], in_=u_sb[64:65, :])
                        # reciprocal on a [128,4] repack (free-size 4, ~20x
                        # cheaper than on a 512-wide row), then broadcast back
                        zpack = small.tile([128, 4], F32, name="zpack", tag="zpack", bufs=2)
                        nc.sync.dma_start(
                            out=zpack,
                            in_=bass.AP(tensor=zb, offset=zoff, ap=[[4, 128], [1, 4]]))
                        rpack = small.tile([128, 4], F32, name="rpack", tag="rpack", bufs=2)
                        nc.vector.reciprocal(rpack, zpack)
                        nc.sync.dma_start(
                            out=bass.AP(tensor=zb2, offset=zoff, ap=[[4, 128], [1, 4]]),
                            in_=rpack)
                        zrep = small.tile([64, 512], F32, name="zrep", tag="zrep", bufs=2)
                        nc.sync.dma_start(
                            out=zrep,
                            in_=bass.AP(tensor=zb2, offset=zoff, ap=[[0, 64], [1, 512]]))
                        nc.gpsimd.tensor_mul(
                            out=vec3[h * 64:(h + 1) * 64, p * 512:(p + 1) * 512, b],
                            in0=u_sb[0:64, :], in1=zrep,
                        )

                # ---- A2A(b): redistribute this batch's vec^T columns ----
                a2a_v = a2a_in[b].rearrange("j p c -> p j c")
                for j in range(NC_):
                    # dest j's rows for batch b: cols (j*256+i)*2 + b
                    nc.sync.dma_start(
                        out=a2a_v[:, j, :],
                        in_=vec3[:, j * RPB:(j + 1) * RPB, b],
                    )
                nc.gpsimd.collective_compute(
                    "AllToAll", mybir.AluOpType.bypass,
                    replica_groups=[list(range(NC_))],
                    ins=[a2a_in[b][:]], outs=[a2a_out[b][:]],
                )
                nc.sync.dma_start(out=vt_b[b], in_=a2a_out[b].rearrange("j p c -> p j c"))

            # ---- P5(b): Wo projection + residual + LayerNorm on local rows ----
            for b in range(BATCH):
                for st in range(RPB // 128):
                    hr = big.tile([128, DM], F32, name="hr", tag="hr", bufs=2)
                    nc.sync.dma_start(out=hr, in_=hrows[b, st * 128:(st + 1) * 128, :])
                    x = big.tile([128, DM], F32, name="x", tag="x", bufs=2)
                    for mh in range(2):
                        ps = mm_ps.tile([128, 512], F32, name="ps", tag="mmps")
                        for c8 in range(NC_):
                            nc.tensor.matmul(
                                ps,
                                lhsT=vt_b[b][:, c8, st * 128:(st + 1) * 128],
                                rhs=wot_sb[:, c8, mh * 512:(mh + 1) * 512],
                                start=(c8 == 0), stop=(c8 == NC_ - 1),
                            )
                        nc.vector.tensor_add(
                            out=x[:, mh * 512:(mh + 1) * 512],
                            in0=ps, in1=hr[:, mh * 512:(mh + 1) * 512],
                        )
                    stats = small.tile([128, 2, 6], F32)
                    nc.vector.bn_stats(out=stats[:, 0, :], in_=x[:, 0:512])
                    nc.vector.bn_stats(out=stats[:, 1, :], in_=x[:, 512:1024])
                    mv = small.tile([128, 2], F32)
                    nc.vector.bn_aggr(out=mv, in_=stats)
                    rstd = small.tile([128, 1], F32)
                    nc.scalar.activation(out=rstd, in_=mv[:, 1:2],
                                         func=mybir.ActivationFunctionType.Sqrt,
                                         bias=eps_sb, scale=1.0)
                    nc.vector.reciprocal(rstd, rstd)
                    nc.vector.tensor_scalar(
                        out=x, in0=x,
                        scalar1=mv[:, 0:1], scalar2=rstd,
                        op0=mybir.AluOpType.subtract, op1=mybir.AluOpType.mult,
                    )
                    nc.vector.tensor_mul(out=x, in0=x, in1=gam_sb)
                    nc.vector.tensor_add(out=x, in0=x, in1=bet_sb)
                    # b-major tile rows i -> interleaved local flat rows 2*i + b
                    nc.sync.dma_start(
                        out=bass.AP(tensor=out, offset=(st * 256 + b) * DM,
                                    ap=[[2 * DM, 128], [1, DM]]),
                        in_=x,
                    )

    _legalize_waits(nc)
    return nc


# ---------------------------------------------------------------------------
# host wrapper
# ---------------------------------------------------------------------------
_CACHE = {}
_partial_mask_tiles = frozenset()


def _get_nc(attn_mask: np.ndarray):
    global _partial_mask_tiles
    masked_full = frozenset(
        (t, b) for t in range(NT) for b in range(BATCH)
        if attn_mask[t * 128:(t + 1) * 128, b].all()
    )
    partial = frozenset(
        (t, b) for t in range(NT) for b in range(BATCH)
        if attn_mask[t * 128:(t + 1) * 128, b].any() and (t, b) not in masked_full
    )
    _partial_mask_tiles = partial
    key = (masked_full, partial)
    if key not in _CACHE:
        _CACHE[key] = _build(masked_full, bool(partial))
    return _CACHE[key]


def _in_maps(h, attn_mask, Wqkv, Wo, gamma, beta):
    h = np.asarray(h, np.float32)
    attn_mask = np.asarray(attn_mask, bool)
    Wqkv = np.asarray(Wqkv, np.float32)
    Wo = np.asarray(Wo, np.float32)
    gamma = np.asarray(gamma, np.float32)
    beta = np.asarray(beta, np.float32)

    hT = np.ascontiguousarray(h.transpose(1, 2, 0)).astype(BF16)      # [B, DM, SEQ]
    woT = np.ascontiguousarray(Wo.T).astype(BF16)                     # [DM(ch), DM(m)]
    mb = np.zeros((128, NT * BATCH), np.float32)
    for t in range(NT):
        for b in range(BATCH):
            mb[:, t * BATCH + b] = np.where(attn_mask[t * 128:(t + 1) * 128, b], NEG, 0.0)

    maps = []
    for c in range(NC_):
        h0, h1 = HPC * c, HPC * c + 1
        rows = []
        for sec in range(3):  # q, k, v
            for hh in (h0, h1):
                rows.append(Wqkv[sec * NH * DH + hh * DH: sec * NH * DH + (hh + 1) * DH])
        wT = np.ascontiguousarray(np.concatenate(rows, 0).T).astype(BF16)  # [DM, 384]
        # b-major residual rows: hrows[b, i] = h[c*256 + i, b, :]
        hrows = np.ascontiguousarray(
            h[RPB * c: RPB * (c + 1)].transpose(1, 0, 2))                  # [B, 256, DM]
        maps.append({
            "hT": hT,
            "wT": wT,
            "woT": woT,
            "hrows": hrows,
            "mb": mb,
            "gamma": gamma,
            "beta": beta,
        })
    return maps


LAST_RES = None


def kernel(h, attn_mask, Wqkv, Wo, gamma, beta, _trace=False):
    global LAST_RES
    nc = _get_nc(np.asarray(attn_mask, bool))
    maps = _in_maps(h, attn_mask, Wqkv, Wo, gamma, beta)
    res = run_bass_kernel_spmd(nc, maps, core_ids=list(range(NC_)), trace=_trace)
    LAST_RES = res
    full = np.concatenate([res.results[c]["out"] for c in range(NC_)], 0)
    out = full.reshape(SEQ, BATCH, DM)
    if _trace:
        return out, res.exec_time_ns
    return out
